# revision 22
# baseline (speedup 1.0000x reference)
"""Multi-head causal attention on 8 Trainium2 cores.

Sharding: core = (batch b in 0..3, head-group g in 0..1). Each core computes
Q/K/V projections for its 8 heads of its batch, causal attention, and a
partial output projection (Wo row-split); host sums the two partials per
batch and transposes back.

Device layout notes (v4 — paired exps, pipelined emission, warm PE):
  - All matmul inputs are bf16 (1 cyc/row on PE, same as fp32r, half SBUF).
  - Q^T, K^T, V stay resident in SBUF between projection and attention.
  - PE warm-up: dummy matmuls bridge the ~8us DMA-dead window at t=0 and
    hold the PE pstate at full clock until x arrives; x streams on two DMA
    queues (sync+gpsimd) to double early bandwidth.
  - Score tiles are [128, 1024] fp32 across two PSUM banks; one EXP
    activation covers a pair of k-tiles (or a packed 896/384 diagonal
    pair), halving the ACT instruction count.
  - Attention emission is software-pipelined depth 2: ctx matmuls of unit
    i are emitted after the scores of unit i+2, so the PE never waits on
    the exp ladder.
  - The softmax-denominator ones-matmul + normalize of head h are deferred
    into head h+1's instruction stream (no PE head-of-line block on the
    DVE acc chain).
  - Half-0's output projection is interleaved inside half-1's attention as
    PE filler; wo is loaded in nt-slices matching consumption order.
"""

import numpy as np
import ml_dtypes

import concourse.bacc as bacc
import concourse.mybir as mybir
import concourse.tile as tile
from concourse.bass_utils import run_bass_kernel_spmd

B, T, D = 4, 2048, 2048
NH, HD = 16, 128
G = 8                       # heads per core
GD = G * HD                 # 1024, group channel width
P = 128
QC = 512                    # q-chunk (PSUM bank width in fp32)
QC2 = 2 * QC
NKT = T // P                # 16 k-tiles over the sequence
NDK = D // P                # 16 k-tiles over d_in
NQC = T // QC               # 4 q-chunks
SCALE = 1.0 / float(np.sqrt(HD))
NEG = -1.0e30
NWARM = 0                   # PE warm-up dummy matmuls (corrupts on HW — see log)
XQ_SPLIT = False            # stream x on two DMA queues (corrupts on HW?)
FILLERS_ON = True           # interleave deferred-proj / outproj fillers
WIDE_ACT = False            # single EXP across PSUM banks (races on HW?)

F32 = mybir.dt.float32
F32R = mybir.dt.float32r
BF16 = mybir.dt.bfloat16
EXP = mybir.ActivationFunctionType.Exp


def build_kernel(debug_dump=False):
    nc = bacc.Bacc("TRN2", target_bir_lowering=False, debug=False, num_devices=8,
                   dynamic_dma_scratch_size=2048)

    xT = nc.dram_tensor("xT", [D, T], BF16, kind="ExternalInput")
    # pre-tiled on host: wq/wk [head, p, ko, d], wv [dchunk, p, ko, c]
    wqT = nc.dram_tensor("wqT", [G, P, NDK, HD], BF16, kind="ExternalInput")
    wkT = nc.dram_tensor("wkT", [G, P, NDK, HD], BF16, kind="ExternalInput")
    wvT = nc.dram_tensor("wvT", [2, P, NDK, QC], BF16, kind="ExternalInput")
    # wo pre-tiled nt-major: [nt, p, hh, 128]
    woT = nc.dram_tensor("woT", [NDK, P, G, P], BF16, kind="ExternalInput")
    # triangle mask: NEG where partition (k) > column (q) within a 128 block
    maskadd = nc.dram_tensor("maskadd", [P, P], F32, kind="ExternalInput")
    outT = nc.dram_tensor("outT", [D, T], BF16, kind="ExternalOutput")
    if debug_dump:
        qtD = nc.dram_tensor("qtD", [P, G, T], BF16, kind="ExternalOutput")
        ktD = nc.dram_tensor("ktD", [P, G, T], BF16, kind="ExternalOutput")
        vtD = nc.dram_tensor("vtD", [P, NKT, G, HD], BF16, kind="ExternalOutput")
        c2D = nc.dram_tensor("c2D", [P, 2, G, QC], BF16, kind="ExternalOutput")
        mkD = nc.dram_tensor("mkD", [P, P], F32, kind="ExternalOutput")
        onD = nc.dram_tensor("onD", [P, P // 2], F32, kind="ExternalOutput")
        c3D = nc.dram_tensor("c3D", [P, 2, G, QC], BF16, kind="ExternalOutput")

    xT_t = xT.rearrange("(ko p) t -> p ko t", p=P)
    outT_t = outT.rearrange("(no p) t -> p no t", p=P)

    with tile.TileContext(nc) as tc:
        with (
            tc.tile_pool(name="const", bufs=1) as constp,
            tc.tile_pool(name="kvq", bufs=1) as kvqp,
            tc.tile_pool(name="c2p0", bufs=1) as c2p0,
        ):
            ones_sb = constp.tile([P, P], BF16)
            nc.vector.memset(ones_sb, 1.0)
            mask_sb = constp.tile([P, P], F32)

            kt_sb = kvqp.tile([P, G, T], BF16)           # K^T per head
            qt_sb = kvqp.tile([P, G, T], BF16)           # Q^T per head
            vt_sb = kvqp.tile([P, NKT, G, HD], BF16)     # V per head
            ctx2_0 = c2p0.tile([P, 2, G, QC], BF16)      # half-0 attn output

            oidx = [0]

            def qk_copy(dst, h, c, ps, eng):
                if eng == 0:
                    nc.scalar.copy(dst[:, h, c * QC:(c + 1) * QC], ps)
                else:
                    nc.vector.tensor_copy(dst[:, h, c * QC:(c + 1) * QC], ps)

            with tc.tile_pool(name="xpool", bufs=1) as xpool:
                xt_sb = xpool.tile([P, NDK, T], BF16)    # 8 MB, resident

                # ---------------- A1: Q/K projections ----------------
                with (
                    tc.tile_pool(name="w1pool", bufs=2) as w1p,
                    tc.tile_pool(name="warmp", bufs=1) as warmp,
                    tc.tile_pool(name="psA1", bufs=1, space="PSUM") as psA1,
                ):
                    # PE warm-up: data-free matmuls fill the DMA-dead start
                    # window and keep the pstate maxed until x arrives
                    warm = warmp.tile([P, QC], BF16)
                    nc.vector.memset(warm, 0.0)
                    for i in range(NWARM if NWARM else 0):
                        pw = psA1.tile([P, QC], F32, tag=f"q{i % 4}",
                                       name="pswarm")
                        nc.tensor.matmul(pw, ones_sb, warm, start=True,
                                         stop=True)

                    # first head's weights before the x stream; the first
                    # few k-slices land as small DMAs so matmul k=0 can
                    # start as early as possible
                    wq_sb = w1p.tile([P, NDK, HD], BF16, tag="wq")
                    for kk in (slice(0, 2), slice(2, 8), slice(8, NDK)):
                        nc.scalar.dma_start(wq_sb[:, kk], wqT[0, :, kk])
                    wk_sb = w1p.tile([P, NDK, HD], BF16, tag="wk")
                    for kk in (slice(0, 2), slice(2, 8), slice(8, NDK)):
                        nc.scalar.dma_start(wk_sb[:, kk], wkT[0, :, kk])
                    nc.scalar.dma_start(mask_sb, maskadd[:])

                    # x streams on two queues (even k on sync, odd on
                    # gpsimd) so early bandwidth is not single-queue bound
                    xq = [nc.sync, nc.gpsimd] if XQ_SPLIT else [nc.sync, nc.sync]
                    for k in range(NDK):
                        q = xq[k % 2]
                        if k < 4:
                            for cc in range(4):
                                q.dma_start(
                                    xt_sb[:, k, cc * QC:(cc + 1) * QC],
                                    xT_t[:, k, cc * QC:(cc + 1) * QC])
                        else:
                            q.dma_start(xt_sb[:, k], xT_t[:, k])

                    for h in range(G):
                        full = h < 6
                        if h > 0:
                            wq_sb = w1p.tile([P, NDK, HD], BF16, tag="wq")
                            nc.scalar.dma_start(wq_sb, wqT[h])
                            wk_sb = w1p.tile([P, NDK, HD], BF16, tag="wk")
                            nc.scalar.dma_start(wk_sb, wkT[h])
                        # group 1: Q all chunks (c01 for h6/h7) + K c0,c1.
                        # h0 takes all 8 banks so PE stays saturated while
                        # the x slices stream in
                        qcs = (0, 1, 2, 3) if full else (0, 1)
                        kcs = (0, 1, 2, 3) if h == 0 else (0, 1)
                        psq = {c: psA1.tile([P, QC], F32, tag=f"q{c}", name=f"psq{c}")
                               for c in qcs}
                        psk = {c: psA1.tile([P, QC], F32, tag=f"k{c}", name=f"psk{c}")
                               for c in kcs}
                        for k in range(NDK):
                            st, sp = (k == 0), (k == NDK - 1)
                            for c in qcs:
                                nc.tensor.matmul(
                                    psq[c], wq_sb[:, k],
                                    xt_sb[:, k, c * QC:(c + 1) * QC],
                                    start=st, stop=sp)
                            for c in kcs:
                                nc.tensor.matmul(
                                    psk[c], wk_sb[:, k],
                                    xt_sb[:, k, c * QC:(c + 1) * QC],
                                    start=st, stop=sp)
                        for i, c in enumerate(qcs):
                            qk_copy(qt_sb, h, c, psq[c], i % 2)
                        for i, c in enumerate(kcs):
                            qk_copy(kt_sb, h, c, psk[c], (i + 1) % 2)
                        # group 2: K c2,c3 (full heads) — drains while group-1
                        # copies free their banks
                        if full and h != 0:
                            psk2 = {c: psA1.tile([P, QC], F32, tag=f"k{c}", name=f"psk2{c}")
                                    for c in (2, 3)}
                            for k in range(NDK):
                                for c in (2, 3):
                                    nc.tensor.matmul(
                                        psk2[c], wk_sb[:, k],
                                        xt_sb[:, k, c * QC:(c + 1) * QC],
                                        start=(k == 0), stop=(k == NDK - 1))
                            qk_copy(kt_sb, h, 2, psk2[2], 0)
                            qk_copy(kt_sb, h, 3, psk2[3], 1)

                # ---------------- A2: V projection (dc-split) ----------------
                # wv streams per-k so the V k-loop starts early; two deferred
                # h6 c2/c3 projections fill the PE while the first slices land
                for dc in range(2):
                    with (
                        tc.tile_pool(name=f"wv{dc}", bufs=1) as wvp,
                        tc.tile_pool(name=f"wA2{dc}", bufs=1) as wA2p,
                        tc.tile_pool(name=f"psV{dc}", bufs=2,
                                     space="PSUM") as psV,
                    ):
                        wv_sb = wvp.tile([P, NDK, QC], BF16)
                        for k0 in range(0, NDK, 4):
                            nc.scalar.dma_start(wv_sb[:, k0:k0 + 4],
                                                wvT[dc, :, k0:k0 + 4])
                        wA2 = wA2p.tile([P, NDK, HD], BF16)
                        nc.sync.dma_start(wA2, (wqT if dc == 0 else wkT)[6])
                        for c in (2, 3):
                            ps = psV.tile([P, QC], F32, tag="def", bufs=1)
                            for k in range(NDK):
                                nc.tensor.matmul(
                                    ps, wA2[:, k],
                                    xt_sb[:, k, c * QC:(c + 1) * QC],
                                    start=(k == 0), stop=(k == NDK - 1))
                            dst = qt_sb if dc == 0 else kt_sb
                            nc.vector.tensor_copy(
                                dst[:, 6, c * QC:(c + 1) * QC], ps)
                        for ts in range(NKT):
                            ps = psV.tile([P, QC], F32, tag="v")
                            for k in range(NDK):
                                nc.tensor.matmul(
                                    ps, xt_sb[:, k, ts * P:(ts + 1) * P],
                                    wv_sb[:, k],
                                    start=(k == 0), stop=(k == NDK - 1))
                            nc.vector.tensor_copy(
                                vt_sb[:, ts, 4 * dc:4 * (dc + 1), :],
                                ps.rearrange("p (g c) -> p g c", g=4))

                # ---------------- overlap: half-0 attention + deferred
                # c2/c3 projections of h6/h7 as PE filler ----------------
                with (
                    tc.tile_pool(name="w2pool", bufs=2) as w2p,
                    tc.tile_pool(name="pp0", bufs=5) as pp0,
                    tc.tile_pool(name="prp0", bufs=2) as prp0,
                    tc.tile_pool(name="accp0", bufs=2) as accp0,
                    tc.tile_pool(name="izp0", bufs=1) as izp0,
                    tc.tile_pool(name="psS0", bufs=2, space="PSUM") as psS0,
                    tc.tile_pool(name="psC0", bufs=1, space="PSUM") as psC0,
                    tc.tile_pool(name="psZ0", bufs=1, space="PSUM") as psZ0,
                ):
                    # deferred unit list: grouped so one w tile serves 2 units
                    defer = [(wt, 7, c) for wt in (0, 1) for c in (2, 3)]
                    dstate = {"i": 0, "w": None}

                    def defer_w_load(gi):
                        wt, h, _ = defer[2 * gi]
                        w = w2p.tile([P, NDK, HD], BF16, tag="w2")
                        nc.scalar.dma_start(w, (wqT if wt == 0 else wkT)[h])
                        return w

                    dstate["w"] = defer_w_load(0)
                    dstate["wnext"] = None

                    def filler0():
                        i = dstate["i"]
                        if i >= len(defer):
                            return
                        wt, h, c = defer[i]
                        if i % 2 == 0 and i + 2 < len(defer):
                            dstate["wnext"] = defer_w_load((i + 2) // 2)
                        w_sb = dstate["w"]
                        ps = psZ0.tile([P, QC], F32, tag="def", name="psdef")
                        for k in range(NDK):
                            nc.tensor.matmul(
                                ps, w_sb[:, k],
                                xt_sb[:, k, c * QC:(c + 1) * QC],
                                start=(k == 0), stop=(k == NDK - 1))
                        dst = qt_sb if wt == 0 else kt_sb
                        nc.vector.tensor_copy(
                            dst[:, h, c * QC:(c + 1) * QC], ps)
                        if i % 2 == 1:
                            dstate["w"] = dstate["wnext"]
                        dstate["i"] = i + 1

                    pending0 = []
                    for h in range(G):
                        attn_head(nc, h, (0, 1), ctx2_0, psC0, psS0, psZ0,
                                  pp0, prp0, accp0, izp0,
                                  kt_sb, qt_sb, vt_sb, ones_sb, mask_sb,
                                  pending0,
                                  filler=filler0 if FILLERS_ON else None,
                                  fill_budget=1 if h % 2 == 0 else 0)
                    for fn in pending0:
                        fn()
                    pending0.clear()
                    while dstate["i"] < len(defer):
                        filler0()

            if debug_dump:
                nc.sync.dma_start(mkD[:], mask_sb)
                nc.sync.dma_start(onD[:], ones_sb.bitcast(F32))
                for h_ in range(G):
                    nc.sync.dma_start(qtD[:, h_], qt_sb[:, h_])
                    nc.sync.dma_start(ktD[:, h_], kt_sb[:, h_])
                    nc.sync.dma_start(c2D[:, 0, h_], ctx2_0[:, 0, h_])
                    nc.sync.dma_start(c2D[:, 1, h_], ctx2_0[:, 1, h_])
                for ts_ in range(NKT):
                    nc.gpsimd.dma_start(vtD[:, ts_], vt_sb[:, ts_])

            # x freed; half-1 attention + both output projections
            with (
                tc.tile_pool(name="wopool", bufs=1) as wop,
                tc.tile_pool(name="c2p1", bufs=1) as c2p1,
                tc.tile_pool(name="pp1", bufs=5) as pp1,
                tc.tile_pool(name="prp1", bufs=2) as prp1,
                tc.tile_pool(name="accp1", bufs=2) as accp1,
                tc.tile_pool(name="izp1", bufs=1) as izp1,
                tc.tile_pool(name="opool", bufs=3) as op_,
                tc.tile_pool(name="psS1", bufs=2, space="PSUM") as psS1,
                tc.tile_pool(name="psC1", bufs=1, space="PSUM") as psC1,
                tc.tile_pool(name="psZO", bufs=1, space="PSUM") as psZO,
            ):
                ctx2_1 = c2p1.tile([P, 2, G, QC], BF16)
                wo_sb = wop.tile([P, NDK, G, P], BF16)   # 4 MB, nt-major
                woq = [nc.sync, nc.gpsimd]
                for nt in range(NDK):
                    woq[nt % 2].dma_start(wo_sb[:, nt], woT[nt])

                ostate = {"i": 0}
                otiles = [(nt, ci, ci, ctx2_0) for nt in range(NDK)
                          for ci in range(2)]

                def outproj_tile(nt, ci, c, ctx2src, final=False):
                    if final:
                        o_ps = psC1.tile([P, QC], F32,
                                         tag=f"ctx{oidx[0] % 2}",
                                         name="o_ps")
                    else:
                        o_ps = psZO.tile([P, QC], F32, tag="o", name="o_ps")
                    for hh in range(G):
                        nc.tensor.matmul(
                            o_ps, wo_sb[:, nt, hh], ctx2src[:, ci, hh],
                            start=(hh == 0), stop=(hh == G - 1))
                    o_sb = op_.tile([P, QC], BF16, tag="osb")
                    nc.vector.tensor_copy(o_sb, o_ps)
                    nc.sync.dma_start(
                        outT_t[:, nt, c * QC:(c + 1) * QC], o_sb)
                    oidx[0] += 1

                def filler1():
                    i = ostate["i"]
                    if i >= len(otiles):
                        return
                    outproj_tile(*otiles[i])
                    ostate["i"] = i + 1

                # a few outproj tiles up front to cover the phase
                # transition before head 0's exp ladder warms up
                pending1 = []
                for h in range(G):
                    attn_head(nc, h, (2, 3), ctx2_1, psC1, psS1, psZO,
                              pp1, prp1, accp1, izp1,
                              kt_sb, qt_sb, vt_sb, ones_sb, mask_sb,
                              pending1,
                              filler=filler1 if FILLERS_ON else None,
                              fill_budget=(5, 5, 5, 5, 3, 3, 3, 3)[h])
                for fn in pending1:
                    fn()
                pending1.clear()
                if debug_dump:
                    for h_ in range(G):
                        nc.gpsimd.dma_start(c3D[:, 0, h_], ctx2_1[:, 0, h_])
                        nc.gpsimd.dma_start(c3D[:, 1, h_], ctx2_1[:, 1, h_])
                for nt in range(NDK):
                    for ci in range(2):
                        outproj_tile(nt, ci, 2 + ci, ctx2_1, final=True)

    nc.finalize()
    return nc


PIPE_DEPTH = 3              # ctx matmuls trail scores by this many units
DEFER_ZW = True             # denominator matmul deferred into next head


def attn_head(nc, h, c_pair, ctx2, psC, psS, psZ, pp, prp, accp, izp,
              kt_sb, qt_sb, vt_sb, ones_sb, mask_sb, pending,
              filler=None, fill_budget=0):
    """Causal attention for head h over q-chunks c_pair.

    Per chunk the k-tiles are emitted as units: full pairs (two 512-wide
    score matmuls into one 2-bank PSUM tile, one 1024-wide EXP), then a
    packed diagonal pair (512+384 -> one 896-wide EXP) and a second packed
    pair (256+128 -> one 384-wide EXP). Emission is software-pipelined:
    ctx matmuls of unit i are emitted after the scores of unit i+2. The
    denominator ones-matmul + normalize are appended to `pending` and
    emitted inside the next head's stream.
    """
    budget = [fill_budget if filler is not None else 0]

    def fill():
        if budget[0] > 0:
            filler()
            budget[0] -= 1

    unit_no = [0]           # head-global unit counter for pending stagger
    for ci, c in enumerate(c_pair):
        acc = accp.tile([P, QC], BF16, tag=f"acc{ci}")
        ctx_ps = psC.tile([P, QC], F32, tag=f"ctx{ci}")
        qs = qt_sb[:, h, c * QC:(c + 1) * QC]
        nd = 4 * c  # number of full (non-diagonal) k-tiles

        # ---- unit emitters: phase A = scores+exp+acc, phase B = ctx ----
        def mk_pair(kt2):
            def phase_a():
                s2 = psS.tile([P, QC2], F32, tag="s", name="s2")
                nc.tensor.matmul(s2[:, 0:QC],
                                 kt_sb[:, h, kt2 * P:(kt2 + 1) * P], qs,
                                 start=True, stop=True)
                nc.tensor.matmul(s2[:, QC:QC2],
                                 kt_sb[:, h, (kt2 + 1) * P:(kt2 + 2) * P],
                                 qs, start=True, stop=True)
                p2 = pp.tile([P, QC2], BF16, tag="p", name="p2")
                if WIDE_ACT:
                    nc.scalar.activation(p2, s2, EXP, scale=SCALE)
                else:
                    nc.scalar.activation(p2[:, 0:QC], s2[:, 0:QC], EXP,
                                         scale=SCALE)
                    nc.scalar.activation(p2[:, QC:QC2], s2[:, QC:QC2], EXP,
                                         scale=SCALE)
                if kt2 == 0:
                    nc.vector.tensor_add(acc, p2[:, 0:QC], p2[:, QC:QC2])
                else:
                    pr = prp.tile([P, QC], BF16, tag="pr")
                    nc.vector.tensor_add(pr, p2[:, 0:QC], p2[:, QC:QC2])
                    nc.vector.tensor_add(acc, acc, pr)
                return p2

            def phase_b(p2):
                nc.tensor.matmul(ctx_ps, vt_sb[:, kt2, h], p2[:, 0:QC],
                                 start=(kt2 == 0), stop=False)
                nc.tensor.matmul(ctx_ps, vt_sb[:, kt2 + 1, h],
                                 p2[:, QC:QC2], start=False, stop=False)
            return phase_a, phase_b

        def mk_diag_a():
            # j=0 (512 wide) and j=1 (384 wide) packed in one 2-bank tile
            def phase_a():
                sA = psS.tile([P, QC2], F32, tag="s", name="sA")
                nc.tensor.matmul(sA[:, 0:QC],
                                 kt_sb[:, h, nd * P:(nd + 1) * P], qs,
                                 start=True, stop=True)
                nc.tensor.matmul(sA[:, QC:QC + 384],
                                 kt_sb[:, h, (nd + 1) * P:(nd + 2) * P],
                                 qs[:, P:QC], start=True, stop=True)
                nc.vector.tensor_add(sA[:, 0:P], sA[:, 0:P], mask_sb)
                nc.vector.tensor_add(sA[:, QC:QC + P], sA[:, QC:QC + P],
                                     mask_sb)
                pA = pp.tile([P, QC2], BF16, tag="p", name="pA")
                if WIDE_ACT:
                    nc.scalar.activation(pA[:, 0:QC + 384], sA[:, 0:QC + 384],
                                         EXP, scale=SCALE)
                else:
                    nc.scalar.activation(pA[:, 0:QC], sA[:, 0:QC], EXP,
                                         scale=SCALE)
                    nc.scalar.activation(pA[:, QC:QC + 384],
                                         sA[:, QC:QC + 384], EXP, scale=SCALE)
                if nd == 0:
                    nc.vector.tensor_copy(acc, pA[:, 0:QC])
                else:
                    nc.vector.tensor_add(acc, acc, pA[:, 0:QC])
                nc.vector.tensor_add(acc[:, P:QC], acc[:, P:QC],
                                     pA[:, QC:QC + 384])
                return pA

            def phase_b(pA):
                nc.tensor.matmul(ctx_ps, vt_sb[:, nd, h], pA[:, 0:QC],
                                 start=(nd == 0), stop=False)
                nc.tensor.matmul(ctx_ps[:, P:QC], vt_sb[:, nd + 1, h],
                                 pA[:, QC:QC + 384], start=False, stop=False)
            return phase_a, phase_b

        def mk_diag_b():
            # j=2 (256 wide) and j=3 (128 wide) packed in one bank
            def phase_a():
                sB = psS.tile([P, QC2], F32, tag="s", name="sB")
                nc.tensor.matmul(sB[:, 0:2 * P],
                                 kt_sb[:, h, (nd + 2) * P:(nd + 3) * P],
                                 qs[:, 2 * P:QC], start=True, stop=True)
                nc.tensor.matmul(sB[:, 2 * P:3 * P],
                                 kt_sb[:, h, (nd + 3) * P:(nd + 4) * P],
                                 qs[:, 3 * P:QC], start=True, stop=True)
                nc.vector.tensor_add(sB[:, 0:P], sB[:, 0:P], mask_sb)
                nc.vector.tensor_add(sB[:, 2 * P:3 * P], sB[:, 2 * P:3 * P],
                                     mask_sb)
                pB = pp.tile([P, QC2], BF16, tag="p", name="pB")
                nc.scalar.activation(pB[:, 0:3 * P], sB[:, 0:3 * P],
                                     EXP, scale=SCALE)
                nc.vector.tensor_add(acc[:, 2 * P:QC], acc[:, 2 * P:QC],
                                     pB[:, 0:2 * P])
                nc.vector.tensor_add(acc[:, 3 * P:QC], acc[:, 3 * P:QC],
                                     pB[:, 2 * P:3 * P])
                return pB

            def phase_b(pB):
                nc.tensor.matmul(ctx_ps[:, 2 * P:QC], vt_sb[:, nd + 2, h],
                                 pB[:, 0:2 * P], start=False, stop=False)
                nc.tensor.matmul(ctx_ps[:, 3 * P:QC], vt_sb[:, nd + 3, h],
                                 pB[:, 2 * P:3 * P], start=False, stop=True)
            return phase_a, phase_b

        units = [mk_pair(kt2) for kt2 in range(0, nd, 2)]
        units.append(mk_diag_a())
        units.append(mk_diag_b())

        # ---- pipelined emission ----
        inflight = []           # [(phase_b, p_tile), ...]
        for ui, (pa, pb) in enumerate(units):
            p_t = pa()
            inflight.append((pb, p_t))
            if unit_no[0] >= 1 and pending:
                # flush one deferred denominator unit of the previous
                # head behind our scores; its DVE chain has drained
                pending.pop(0)()
            unit_no[0] += 1
            if len(inflight) > PIPE_DEPTH:
                fb, ft = inflight.pop(0)
                fb(ft)
            if unit_no[0] % 2 == 0:
                fill()
        for fb, ft in inflight:
            fb(ft)
        fill()

        def mk_pending(ci, acc, ctx_ps):
            def flushfn():
                zw = psZ.tile([P, QC], F32, tag="z", name="zw")
                nc.tensor.matmul(zw, ones_sb, acc, start=True, stop=True)
                iz = izp.tile([P, QC], F32, tag=f"iz{ci}")
                nc.vector.reciprocal_approx_fast(iz, zw)
                nc.vector.tensor_mul(ctx2[:, ci, h], ctx_ps, iz)
            return flushfn
        if DEFER_ZW:
            pending.append(mk_pending(ci, acc, ctx_ps))
        else:
            mk_pending(ci, acc, ctx_ps)()
    while budget[0] > 0:
        fill()


_NC = None
DEBUG_NC = False


def _get_nc():
    global _NC
    if _NC is None:
        _NC = build_kernel(debug_dump=DEBUG_NC)
    return _NC


def _make_mask():
    m = np.zeros((P, P), dtype=np.float32)
    i = np.arange(P)[:, None]
    col = np.arange(P)[None, :]
    m[i > col] = NEG
    return m


def kernel(x, Wq, Wk, Wv, Wo, _trace=False, _trace_kwargs=None):
    bf16 = ml_dtypes.bfloat16
    x = np.asarray(x, dtype=np.float32)
    Wq = np.asarray(Wq, dtype=np.float32)
    Wk = np.asarray(Wk, dtype=np.float32)
    Wv = np.asarray(Wv, dtype=np.float32)
    Wo = np.asarray(Wo, dtype=np.float32)

    nc = _get_nc()
    mask = _make_mask()

    # [d_out, d_in] -> [h, p, ko, dd] tiles per head-group chunk of 8 heads
    def tile_qk(W, g):
        wt = W.T[:, g * GD:(g + 1) * GD]              # [D, GD]
        return np.ascontiguousarray(
            wt.reshape(NDK, P, G, HD).transpose(2, 1, 0, 3).astype(bf16))

    def tile_v(W, g):
        wt = W.T[:, g * GD:(g + 1) * GD]              # [D, GD]
        return np.ascontiguousarray(
            wt.reshape(NDK, P, 2, QC).transpose(2, 1, 0, 3).astype(bf16))

    def tile_wo(W, g):
        wt = W.T[g * GD:(g + 1) * GD, :]              # [GD, D]
        # [nt, p(of head block), hh, 128]
        return np.ascontiguousarray(
            wt.reshape(G, P, NDK, P).transpose(2, 1, 0, 3).astype(bf16))

    in_maps = []
    for core in range(8):
        b, g = divmod(core, 2)
        in_maps.append({
            "xT": np.ascontiguousarray(x[b].T.astype(bf16)),
            "wqT": tile_qk(Wq, g),
            "wkT": tile_qk(Wk, g),
            "wvT": tile_v(Wv, g),
            "woT": tile_wo(Wo, g),
            "maskadd": mask,
        })

    kwargs = {}
    if _trace:
        kwargs.update(trace=True, **(_trace_kwargs or {}))
    res = run_bass_kernel_spmd(nc, in_maps, core_ids=list(range(8)), **kwargs)

    out = np.empty((B, T, D), dtype=np.float32)
    for b in range(B):
        acc = (np.asarray(res.results[2 * b]["outT"], dtype=np.float32)
               + np.asarray(res.results[2 * b + 1]["outT"], dtype=np.float32))
        out[b] = acc.T
    if _trace:
        return out, res
    return out


# revision 23
# speedup vs baseline: 1.0265x; 1.0265x over previous
"""Multi-head causal attention on 8 Trainium2 cores.

Sharding: core = (batch b in 0..3, head-group g in 0..1). Each core computes
Q/K/V projections for its 8 heads of its batch, causal attention, and a
partial output projection (Wo row-split); host sums the two partials per
batch and transposes back.

Device layout notes (v4 — paired exps, pipelined emission, warm PE):
  - All matmul inputs are bf16 (1 cyc/row on PE, same as fp32r, half SBUF).
  - Q^T, K^T, V stay resident in SBUF between projection and attention.
  - PE warm-up: dummy matmuls bridge the ~8us DMA-dead window at t=0 and
    hold the PE pstate at full clock until x arrives; x streams on two DMA
    queues (sync+gpsimd) to double early bandwidth.
  - Score tiles are [128, 1024] fp32 across two PSUM banks; one EXP
    activation covers a pair of k-tiles (or a packed 896/384 diagonal
    pair), halving the ACT instruction count.
  - Attention emission is software-pipelined depth 2: ctx matmuls of unit
    i are emitted after the scores of unit i+2, so the PE never waits on
    the exp ladder.
  - The softmax-denominator ones-matmul + normalize of head h are deferred
    into head h+1's instruction stream (no PE head-of-line block on the
    DVE acc chain).
  - Half-0's output projection is interleaved inside half-1's attention as
    PE filler; wo is loaded in nt-slices matching consumption order.
"""

import numpy as np
import ml_dtypes

import concourse.bacc as bacc
import concourse.mybir as mybir
import concourse.tile as tile
from concourse.bass_utils import run_bass_kernel_spmd

B, T, D = 4, 2048, 2048
NH, HD = 16, 128
G = 8                       # heads per core
GD = G * HD                 # 1024, group channel width
P = 128
QC = 512                    # q-chunk (PSUM bank width in fp32)
QC2 = 2 * QC
NKT = T // P                # 16 k-tiles over the sequence
NDK = D // P                # 16 k-tiles over d_in
NQC = T // QC               # 4 q-chunks
SCALE = 1.0 / float(np.sqrt(HD))
NEG = -1.0e30
NWARM = 0                   # PE warm-up dummy matmuls (corrupts on HW — see log)
XQ_SPLIT = False            # stream x on two DMA queues (corrupts on HW?)
FILLERS_ON = True           # interleave deferred-proj / outproj fillers
WIDE_ACT = True             # 1024-wide pair EXPs (bank-aligned, full tiles);
                            # the odd-width 896 diag EXP always stays split

F32 = mybir.dt.float32
F32R = mybir.dt.float32r
BF16 = mybir.dt.bfloat16
EXP = mybir.ActivationFunctionType.Exp


def build_kernel(debug_dump=False):
    nc = bacc.Bacc("TRN2", target_bir_lowering=False, debug=False, num_devices=8,
                   dynamic_dma_scratch_size=2048)

    xT = nc.dram_tensor("xT", [D, T], BF16, kind="ExternalInput")
    # pre-tiled on host: wq/wk [head, p, ko, d], wv [dchunk, p, ko, c]
    wqT = nc.dram_tensor("wqT", [G, P, NDK, HD], BF16, kind="ExternalInput")
    wkT = nc.dram_tensor("wkT", [G, P, NDK, HD], BF16, kind="ExternalInput")
    wvT = nc.dram_tensor("wvT", [2, P, NDK, QC], BF16, kind="ExternalInput")
    # wo pre-tiled nt-major: [nt, p, hh, 128]
    woT = nc.dram_tensor("woT", [NDK, P, G, P], BF16, kind="ExternalInput")
    # triangle mask: NEG where partition (k) > column (q) within a 128 block
    maskadd = nc.dram_tensor("maskadd", [P, P], F32, kind="ExternalInput")
    outT = nc.dram_tensor("outT", [D, T], BF16, kind="ExternalOutput")
    if debug_dump:
        qtD = nc.dram_tensor("qtD", [P, G, T], BF16, kind="ExternalOutput")
        ktD = nc.dram_tensor("ktD", [P, G, T], BF16, kind="ExternalOutput")
        vtD = nc.dram_tensor("vtD", [P, NKT, G, HD], BF16, kind="ExternalOutput")
        c2D = nc.dram_tensor("c2D", [P, 2, G, QC], BF16, kind="ExternalOutput")
        mkD = nc.dram_tensor("mkD", [P, P], F32, kind="ExternalOutput")
        onD = nc.dram_tensor("onD", [P, P // 2], F32, kind="ExternalOutput")
        c3D = nc.dram_tensor("c3D", [P, 2, G, QC], BF16, kind="ExternalOutput")

    xT_t = xT.rearrange("(ko p) t -> p ko t", p=P)
    outT_t = outT.rearrange("(no p) t -> p no t", p=P)

    with tile.TileContext(nc) as tc:
        with (
            tc.tile_pool(name="const", bufs=1) as constp,
            tc.tile_pool(name="kvq", bufs=1) as kvqp,
            tc.tile_pool(name="c2p0", bufs=1) as c2p0,
        ):
            ones_sb = constp.tile([P, P], BF16)
            nc.vector.memset(ones_sb, 1.0)
            mask_sb = constp.tile([P, P], F32)

            kt_sb = kvqp.tile([P, G, T], BF16)           # K^T per head
            qt_sb = kvqp.tile([P, G, T], BF16)           # Q^T per head
            vt_sb = kvqp.tile([P, NKT, G, HD], BF16)     # V per head
            ctx2_0 = c2p0.tile([P, 2, G, QC], BF16)      # half-0 attn output

            oidx = [0]

            def qk_copy(dst, h, c, ps, eng):
                if eng == 0:
                    nc.scalar.copy(dst[:, h, c * QC:(c + 1) * QC], ps)
                else:
                    nc.vector.tensor_copy(dst[:, h, c * QC:(c + 1) * QC], ps)

            with tc.tile_pool(name="xpool", bufs=1) as xpool:
                xt_sb = xpool.tile([P, NDK, T], BF16)    # 8 MB, resident

                # ---------------- A1: Q/K projections ----------------
                with (
                    tc.tile_pool(name="w1pool", bufs=2) as w1p,
                    tc.tile_pool(name="warmp", bufs=1) as warmp,
                    tc.tile_pool(name="psA1", bufs=1, space="PSUM") as psA1,
                ):
                    # PE warm-up: data-free matmuls fill the DMA-dead start
                    # window and keep the pstate maxed until x arrives
                    warm = warmp.tile([P, QC], BF16)
                    nc.vector.memset(warm, 0.0)
                    for i in range(NWARM if NWARM else 0):
                        pw = psA1.tile([P, QC], F32, tag=f"q{i % 4}",
                                       name="pswarm")
                        nc.tensor.matmul(pw, ones_sb, warm, start=True,
                                         stop=True)

                    # first head's weights before the x stream; the first
                    # few k-slices land as small DMAs so matmul k=0 can
                    # start as early as possible
                    wq_sb = w1p.tile([P, NDK, HD], BF16, tag="wq")
                    for kk in (slice(0, 2), slice(2, 8), slice(8, NDK)):
                        nc.scalar.dma_start(wq_sb[:, kk], wqT[0, :, kk])
                    wk_sb = w1p.tile([P, NDK, HD], BF16, tag="wk")
                    for kk in (slice(0, 2), slice(2, 8), slice(8, NDK)):
                        nc.scalar.dma_start(wk_sb[:, kk], wkT[0, :, kk])
                    nc.scalar.dma_start(mask_sb, maskadd[:])

                    # x streams on two queues (even k on sync, odd on
                    # gpsimd) so early bandwidth is not single-queue bound
                    xq = [nc.sync, nc.gpsimd] if XQ_SPLIT else [nc.sync, nc.sync]
                    for k in range(NDK):
                        q = xq[k % 2]
                        if k < 4:
                            for cc in range(4):
                                q.dma_start(
                                    xt_sb[:, k, cc * QC:(cc + 1) * QC],
                                    xT_t[:, k, cc * QC:(cc + 1) * QC])
                        else:
                            q.dma_start(xt_sb[:, k], xT_t[:, k])

                    for h in range(G):
                        full = h < 6
                        if h > 0:
                            wq_sb = w1p.tile([P, NDK, HD], BF16, tag="wq")
                            nc.scalar.dma_start(wq_sb, wqT[h])
                            wk_sb = w1p.tile([P, NDK, HD], BF16, tag="wk")
                            nc.scalar.dma_start(wk_sb, wkT[h])
                        # group 1: Q all chunks (c01 for h6/h7) + K c0,c1.
                        # h0 takes all 8 banks so PE stays saturated while
                        # the x slices stream in
                        qcs = (0, 1, 2, 3) if full else (0, 1)
                        kcs = (0, 1, 2, 3) if h == 0 else (0, 1)
                        psq = {c: psA1.tile([P, QC], F32, tag=f"q{c}", name=f"psq{c}")
                               for c in qcs}
                        psk = {c: psA1.tile([P, QC], F32, tag=f"k{c}", name=f"psk{c}")
                               for c in kcs}
                        for k in range(NDK):
                            st, sp = (k == 0), (k == NDK - 1)
                            for c in qcs:
                                nc.tensor.matmul(
                                    psq[c], wq_sb[:, k],
                                    xt_sb[:, k, c * QC:(c + 1) * QC],
                                    start=st, stop=sp)
                            for c in kcs:
                                nc.tensor.matmul(
                                    psk[c], wk_sb[:, k],
                                    xt_sb[:, k, c * QC:(c + 1) * QC],
                                    start=st, stop=sp)
                        for i, c in enumerate(qcs):
                            qk_copy(qt_sb, h, c, psq[c], i % 2)
                        for i, c in enumerate(kcs):
                            qk_copy(kt_sb, h, c, psk[c], (i + 1) % 2)
                        # group 2: K c2,c3 (full heads) — drains while group-1
                        # copies free their banks
                        if full and h != 0:
                            psk2 = {c: psA1.tile([P, QC], F32, tag=f"k{c}", name=f"psk2{c}")
                                    for c in (2, 3)}
                            for k in range(NDK):
                                for c in (2, 3):
                                    nc.tensor.matmul(
                                        psk2[c], wk_sb[:, k],
                                        xt_sb[:, k, c * QC:(c + 1) * QC],
                                        start=(k == 0), stop=(k == NDK - 1))
                            qk_copy(kt_sb, h, 2, psk2[2], 0)
                            qk_copy(kt_sb, h, 3, psk2[3], 1)

                # ---------------- A2: V projection (dc-split) ----------------
                # wv streams per-k so the V k-loop starts early; two deferred
                # h6 c2/c3 projections fill the PE while the first slices land
                for dc in range(2):
                    with (
                        tc.tile_pool(name=f"wv{dc}", bufs=1) as wvp,
                        tc.tile_pool(name=f"wA2{dc}", bufs=1) as wA2p,
                        tc.tile_pool(name=f"psV{dc}", bufs=2,
                                     space="PSUM") as psV,
                    ):
                        wv_sb = wvp.tile([P, NDK, QC], BF16)
                        for k0 in range(0, NDK, 4):
                            nc.scalar.dma_start(wv_sb[:, k0:k0 + 4],
                                                wvT[dc, :, k0:k0 + 4])
                        wA2 = wA2p.tile([P, NDK, HD], BF16)
                        nc.sync.dma_start(wA2, (wqT if dc == 0 else wkT)[6])
                        for c in (2, 3):
                            ps = psV.tile([P, QC], F32, tag="def", bufs=1)
                            for k in range(NDK):
                                nc.tensor.matmul(
                                    ps, wA2[:, k],
                                    xt_sb[:, k, c * QC:(c + 1) * QC],
                                    start=(k == 0), stop=(k == NDK - 1))
                            dst = qt_sb if dc == 0 else kt_sb
                            nc.vector.tensor_copy(
                                dst[:, 6, c * QC:(c + 1) * QC], ps)
                        for ts in range(NKT):
                            ps = psV.tile([P, QC], F32, tag="v")
                            for k in range(NDK):
                                nc.tensor.matmul(
                                    ps, xt_sb[:, k, ts * P:(ts + 1) * P],
                                    wv_sb[:, k],
                                    start=(k == 0), stop=(k == NDK - 1))
                            nc.vector.tensor_copy(
                                vt_sb[:, ts, 4 * dc:4 * (dc + 1), :],
                                ps.rearrange("p (g c) -> p g c", g=4))

                # ---------------- overlap: half-0 attention + deferred
                # c2/c3 projections of h6/h7 as PE filler ----------------
                with (
                    tc.tile_pool(name="w2pool", bufs=2) as w2p,
                    tc.tile_pool(name="pp0", bufs=5) as pp0,
                    tc.tile_pool(name="prp0", bufs=2) as prp0,
                    tc.tile_pool(name="accp0", bufs=2) as accp0,
                    tc.tile_pool(name="izp0", bufs=1) as izp0,
                    tc.tile_pool(name="psS0", bufs=2, space="PSUM") as psS0,
                    tc.tile_pool(name="psC0", bufs=1, space="PSUM") as psC0,
                    tc.tile_pool(name="psZ0", bufs=1, space="PSUM") as psZ0,
                ):
                    # deferred unit list: grouped so one w tile serves 2 units
                    defer = [(wt, 7, c) for wt in (0, 1) for c in (2, 3)]
                    dstate = {"i": 0, "w": None}

                    def defer_w_load(gi):
                        wt, h, _ = defer[2 * gi]
                        w = w2p.tile([P, NDK, HD], BF16, tag="w2")
                        nc.scalar.dma_start(w, (wqT if wt == 0 else wkT)[h])
                        return w

                    dstate["w"] = defer_w_load(0)
                    dstate["wnext"] = None

                    def filler0():
                        i = dstate["i"]
                        if i >= len(defer):
                            return
                        wt, h, c = defer[i]
                        if i % 2 == 0 and i + 2 < len(defer):
                            dstate["wnext"] = defer_w_load((i + 2) // 2)
                        w_sb = dstate["w"]
                        ps = psZ0.tile([P, QC], F32, tag="def", name="psdef")
                        for k in range(NDK):
                            nc.tensor.matmul(
                                ps, w_sb[:, k],
                                xt_sb[:, k, c * QC:(c + 1) * QC],
                                start=(k == 0), stop=(k == NDK - 1))
                        dst = qt_sb if wt == 0 else kt_sb
                        nc.vector.tensor_copy(
                            dst[:, h, c * QC:(c + 1) * QC], ps)
                        if i % 2 == 1:
                            dstate["w"] = dstate["wnext"]
                        dstate["i"] = i + 1

                    pending0 = []
                    for h in range(G):
                        attn_head(nc, h, (0, 1), ctx2_0, psC0, psS0, psZ0,
                                  pp0, prp0, accp0, izp0,
                                  kt_sb, qt_sb, vt_sb, ones_sb, mask_sb,
                                  pending0,
                                  filler=filler0 if FILLERS_ON else None,
                                  fill_budget=1 if h % 2 == 0 else 0)
                    for fn in pending0:
                        fn()
                    pending0.clear()
                    while dstate["i"] < len(defer):
                        filler0()

            if debug_dump:
                nc.sync.dma_start(mkD[:], mask_sb)
                nc.sync.dma_start(onD[:], ones_sb.bitcast(F32))
                for h_ in range(G):
                    nc.sync.dma_start(qtD[:, h_], qt_sb[:, h_])
                    nc.sync.dma_start(ktD[:, h_], kt_sb[:, h_])
                    nc.sync.dma_start(c2D[:, 0, h_], ctx2_0[:, 0, h_])
                    nc.sync.dma_start(c2D[:, 1, h_], ctx2_0[:, 1, h_])
                for ts_ in range(NKT):
                    nc.gpsimd.dma_start(vtD[:, ts_], vt_sb[:, ts_])

            # x freed; half-1 attention + both output projections
            with (
                tc.tile_pool(name="wopool", bufs=1) as wop,
                tc.tile_pool(name="c2p1", bufs=1) as c2p1,
                tc.tile_pool(name="pp1", bufs=5) as pp1,
                tc.tile_pool(name="prp1", bufs=2) as prp1,
                tc.tile_pool(name="accp1", bufs=2) as accp1,
                tc.tile_pool(name="izp1", bufs=1) as izp1,
                tc.tile_pool(name="opool", bufs=3) as op_,
                tc.tile_pool(name="psS1", bufs=2, space="PSUM") as psS1,
                tc.tile_pool(name="psC1", bufs=1, space="PSUM") as psC1,
                tc.tile_pool(name="psZO", bufs=1, space="PSUM") as psZO,
            ):
                ctx2_1 = c2p1.tile([P, 2, G, QC], BF16)
                wo_sb = wop.tile([P, NDK, G, P], BF16)   # 4 MB, nt-major
                woq = [nc.sync, nc.gpsimd]
                for nt in range(NDK):
                    woq[nt % 2].dma_start(wo_sb[:, nt], woT[nt])

                ostate = {"i": 0}
                otiles = [(nt, ci, ci, ctx2_0) for nt in range(NDK)
                          for ci in range(2)]

                def outproj_tile(nt, ci, c, ctx2src, final=False):
                    if final:
                        o_ps = psC1.tile([P, QC], F32,
                                         tag=f"ctx{oidx[0] % 2}",
                                         name="o_ps")
                    else:
                        o_ps = psZO.tile([P, QC], F32, tag="o", name="o_ps")
                    for hh in range(G):
                        nc.tensor.matmul(
                            o_ps, wo_sb[:, nt, hh], ctx2src[:, ci, hh],
                            start=(hh == 0), stop=(hh == G - 1))
                    o_sb = op_.tile([P, QC], BF16, tag="osb")
                    nc.vector.tensor_copy(o_sb, o_ps)
                    nc.sync.dma_start(
                        outT_t[:, nt, c * QC:(c + 1) * QC], o_sb)
                    oidx[0] += 1

                def filler1():
                    i = ostate["i"]
                    if i >= len(otiles):
                        return
                    outproj_tile(*otiles[i])
                    ostate["i"] = i + 1

                # a few outproj tiles up front to cover the phase
                # transition before head 0's exp ladder warms up
                pending1 = []
                for h in range(G):
                    attn_head(nc, h, (2, 3), ctx2_1, psC1, psS1, psZO,
                              pp1, prp1, accp1, izp1,
                              kt_sb, qt_sb, vt_sb, ones_sb, mask_sb,
                              pending1,
                              filler=filler1 if FILLERS_ON else None,
                              fill_budget=(5, 5, 5, 5, 3, 3, 3, 3)[h])
                for fn in pending1:
                    fn()
                pending1.clear()
                if debug_dump:
                    for h_ in range(G):
                        nc.gpsimd.dma_start(c3D[:, 0, h_], ctx2_1[:, 0, h_])
                        nc.gpsimd.dma_start(c3D[:, 1, h_], ctx2_1[:, 1, h_])
                for nt in range(NDK):
                    for ci in range(2):
                        outproj_tile(nt, ci, 2 + ci, ctx2_1, final=True)

    nc.finalize()
    return nc


PIPE_DEPTH = 3              # ctx matmuls trail scores by this many units
DEFER_ZW = True             # denominator matmul deferred into next head


def attn_head(nc, h, c_pair, ctx2, psC, psS, psZ, pp, prp, accp, izp,
              kt_sb, qt_sb, vt_sb, ones_sb, mask_sb, pending,
              filler=None, fill_budget=0):
    """Causal attention for head h over q-chunks c_pair.

    Per chunk the k-tiles are emitted as units: full pairs (two 512-wide
    score matmuls into one 2-bank PSUM tile, one 1024-wide EXP), then a
    packed diagonal pair (512+384 -> one 896-wide EXP) and a second packed
    pair (256+128 -> one 384-wide EXP). Emission is software-pipelined:
    ctx matmuls of unit i are emitted after the scores of unit i+2. The
    denominator ones-matmul + normalize are appended to `pending` and
    emitted inside the next head's stream.
    """
    budget = [fill_budget if filler is not None else 0]

    def fill():
        if budget[0] > 0:
            filler()
            budget[0] -= 1

    unit_no = [0]           # head-global unit counter for pending stagger
    for ci, c in enumerate(c_pair):
        acc = accp.tile([P, QC], BF16, tag=f"acc{ci}")
        ctx_ps = psC.tile([P, QC], F32, tag=f"ctx{ci}")
        qs = qt_sb[:, h, c * QC:(c + 1) * QC]
        nd = 4 * c  # number of full (non-diagonal) k-tiles

        # ---- unit emitters: phase A = scores+exp+acc, phase B = ctx ----
        def mk_pair(kt2):
            def phase_a():
                s2 = psS.tile([P, QC2], F32, tag="s", name="s2")
                nc.tensor.matmul(s2[:, 0:QC],
                                 kt_sb[:, h, kt2 * P:(kt2 + 1) * P], qs,
                                 start=True, stop=True)
                nc.tensor.matmul(s2[:, QC:QC2],
                                 kt_sb[:, h, (kt2 + 1) * P:(kt2 + 2) * P],
                                 qs, start=True, stop=True)
                p2 = pp.tile([P, QC2], BF16, tag="p", name="p2")
                if WIDE_ACT:
                    nc.scalar.activation(p2, s2, EXP, scale=SCALE)
                else:
                    nc.scalar.activation(p2[:, 0:QC], s2[:, 0:QC], EXP,
                                         scale=SCALE)
                    nc.scalar.activation(p2[:, QC:QC2], s2[:, QC:QC2], EXP,
                                         scale=SCALE)
                if kt2 == 0:
                    nc.vector.tensor_add(acc, p2[:, 0:QC], p2[:, QC:QC2])
                else:
                    pr = prp.tile([P, QC], BF16, tag="pr")
                    nc.vector.tensor_add(pr, p2[:, 0:QC], p2[:, QC:QC2])
                    nc.vector.tensor_add(acc, acc, pr)
                return p2

            def phase_b(p2):
                nc.tensor.matmul(ctx_ps, vt_sb[:, kt2, h], p2[:, 0:QC],
                                 start=(kt2 == 0), stop=False)
                nc.tensor.matmul(ctx_ps, vt_sb[:, kt2 + 1, h],
                                 p2[:, QC:QC2], start=False, stop=False)
            return phase_a, phase_b

        def mk_diag_a():
            # j=0 (512 wide) and j=1 (384 wide) packed in one 2-bank tile
            def phase_a():
                sA = psS.tile([P, QC2], F32, tag="s", name="sA")
                nc.tensor.matmul(sA[:, 0:QC],
                                 kt_sb[:, h, nd * P:(nd + 1) * P], qs,
                                 start=True, stop=True)
                nc.tensor.matmul(sA[:, QC:QC + 384],
                                 kt_sb[:, h, (nd + 1) * P:(nd + 2) * P],
                                 qs[:, P:QC], start=True, stop=True)
                nc.vector.tensor_add(sA[:, 0:P], sA[:, 0:P], mask_sb)
                nc.vector.tensor_add(sA[:, QC:QC + P], sA[:, QC:QC + P],
                                     mask_sb)
                pA = pp.tile([P, QC2], BF16, tag="p", name="pA")
                nc.scalar.activation(pA[:, 0:QC], sA[:, 0:QC], EXP,
                                     scale=SCALE)
                nc.scalar.activation(pA[:, QC:QC + 384],
                                     sA[:, QC:QC + 384], EXP, scale=SCALE)
                if nd == 0:
                    nc.vector.tensor_copy(acc, pA[:, 0:QC])
                else:
                    nc.vector.tensor_add(acc, acc, pA[:, 0:QC])
                nc.vector.tensor_add(acc[:, P:QC], acc[:, P:QC],
                                     pA[:, QC:QC + 384])
                return pA

            def phase_b(pA):
                nc.tensor.matmul(ctx_ps, vt_sb[:, nd, h], pA[:, 0:QC],
                                 start=(nd == 0), stop=False)
                nc.tensor.matmul(ctx_ps[:, P:QC], vt_sb[:, nd + 1, h],
                                 pA[:, QC:QC + 384], start=False, stop=False)
            return phase_a, phase_b

        def mk_diag_b():
            # j=2 (256 wide) and j=3 (128 wide) packed in one bank
            def phase_a():
                sB = psS.tile([P, QC2], F32, tag="s", name="sB")
                nc.tensor.matmul(sB[:, 0:2 * P],
                                 kt_sb[:, h, (nd + 2) * P:(nd + 3) * P],
                                 qs[:, 2 * P:QC], start=True, stop=True)
                nc.tensor.matmul(sB[:, 2 * P:3 * P],
                                 kt_sb[:, h, (nd + 3) * P:(nd + 4) * P],
                                 qs[:, 3 * P:QC], start=True, stop=True)
                nc.vector.tensor_add(sB[:, 0:P], sB[:, 0:P], mask_sb)
                nc.vector.tensor_add(sB[:, 2 * P:3 * P], sB[:, 2 * P:3 * P],
                                     mask_sb)
                pB = pp.tile([P, QC2], BF16, tag="p", name="pB")
                nc.scalar.activation(pB[:, 0:3 * P], sB[:, 0:3 * P],
                                     EXP, scale=SCALE)
                nc.vector.tensor_add(acc[:, 2 * P:QC], acc[:, 2 * P:QC],
                                     pB[:, 0:2 * P])
                nc.vector.tensor_add(acc[:, 3 * P:QC], acc[:, 3 * P:QC],
                                     pB[:, 2 * P:3 * P])
                return pB

            def phase_b(pB):
                nc.tensor.matmul(ctx_ps[:, 2 * P:QC], vt_sb[:, nd + 2, h],
                                 pB[:, 0:2 * P], start=False, stop=False)
                nc.tensor.matmul(ctx_ps[:, 3 * P:QC], vt_sb[:, nd + 3, h],
                                 pB[:, 2 * P:3 * P], start=False, stop=True)
            return phase_a, phase_b

        units = [mk_pair(kt2) for kt2 in range(0, nd, 2)]
        units.append(mk_diag_a())
        units.append(mk_diag_b())

        # ---- pipelined emission ----
        inflight = []           # [(phase_b, p_tile), ...]
        for ui, (pa, pb) in enumerate(units):
            p_t = pa()
            inflight.append((pb, p_t))
            if unit_no[0] >= 1 and pending:
                # flush one deferred denominator unit of the previous
                # head behind our scores; its DVE chain has drained
                pending.pop(0)()
            unit_no[0] += 1
            if len(inflight) > PIPE_DEPTH:
                fb, ft = inflight.pop(0)
                fb(ft)
            if unit_no[0] % 2 == 0:
                fill()
        for fb, ft in inflight:
            fb(ft)
        fill()

        def mk_pending(ci, acc, ctx_ps):
            def flushfn():
                zw = psZ.tile([P, QC], F32, tag="z", name="zw")
                nc.tensor.matmul(zw, ones_sb, acc, start=True, stop=True)
                iz = izp.tile([P, QC], F32, tag=f"iz{ci}")
                nc.vector.reciprocal_approx_fast(iz, zw)
                nc.vector.tensor_mul(ctx2[:, ci, h], ctx_ps, iz)
            return flushfn
        if DEFER_ZW:
            pending.append(mk_pending(ci, acc, ctx_ps))
        else:
            mk_pending(ci, acc, ctx_ps)()
    while budget[0] > 0:
        fill()


_NC = None
DEBUG_NC = False


def _get_nc():
    global _NC
    if _NC is None:
        _NC = build_kernel(debug_dump=DEBUG_NC)
    return _NC


def _make_mask():
    m = np.zeros((P, P), dtype=np.float32)
    i = np.arange(P)[:, None]
    col = np.arange(P)[None, :]
    m[i > col] = NEG
    return m


def kernel(x, Wq, Wk, Wv, Wo, _trace=False, _trace_kwargs=None):
    bf16 = ml_dtypes.bfloat16
    x = np.asarray(x, dtype=np.float32)
    Wq = np.asarray(Wq, dtype=np.float32)
    Wk = np.asarray(Wk, dtype=np.float32)
    Wv = np.asarray(Wv, dtype=np.float32)
    Wo = np.asarray(Wo, dtype=np.float32)

    nc = _get_nc()
    mask = _make_mask()

    # [d_out, d_in] -> [h, p, ko, dd] tiles per head-group chunk of 8 heads
    def tile_qk(W, g):
        wt = W.T[:, g * GD:(g + 1) * GD]              # [D, GD]
        return np.ascontiguousarray(
            wt.reshape(NDK, P, G, HD).transpose(2, 1, 0, 3).astype(bf16))

    def tile_v(W, g):
        wt = W.T[:, g * GD:(g + 1) * GD]              # [D, GD]
        return np.ascontiguousarray(
            wt.reshape(NDK, P, 2, QC).transpose(2, 1, 0, 3).astype(bf16))

    def tile_wo(W, g):
        wt = W.T[g * GD:(g + 1) * GD, :]              # [GD, D]
        # [nt, p(of head block), hh, 128]
        return np.ascontiguousarray(
            wt.reshape(G, P, NDK, P).transpose(2, 1, 0, 3).astype(bf16))

    in_maps = []
    for core in range(8):
        b, g = divmod(core, 2)
        in_maps.append({
            "xT": np.ascontiguousarray(x[b].T.astype(bf16)),
            "wqT": tile_qk(Wq, g),
            "wkT": tile_qk(Wk, g),
            "wvT": tile_v(Wv, g),
            "woT": tile_wo(Wo, g),
            "maskadd": mask,
        })

    kwargs = {}
    if _trace:
        kwargs.update(trace=True, **(_trace_kwargs or {}))
    res = run_bass_kernel_spmd(nc, in_maps, core_ids=list(range(8)), **kwargs)

    out = np.empty((B, T, D), dtype=np.float32)
    for b in range(B):
        acc = (np.asarray(res.results[2 * b]["outT"], dtype=np.float32)
               + np.asarray(res.results[2 * b + 1]["outT"], dtype=np.float32))
        out[b] = acc.T
    if _trace:
        return out, res
    return out
